# revision 4
# baseline (speedup 1.0000x reference)
"""Episode-parallel meta-learning classifier on 8 Trainium2 NeuronCores.

For each of E=4000 independent episodes: train a tiny MLP (64 -> 128 -> 5)
for 10 SGD steps on S=25 support examples (softmax cross-entropy), then
evaluate Q=75 queries.  Output: [E*Q, WAY] = [300000, 5] float32.

Strategy: pure episode parallelism — 500 episodes per core, one Bass/Tile
kernel per core (SPMD over 8 cores via shard_map/PJRT), single NEFF for the
whole computation.  The inner loop is restructured so W1 never enters it:
track z1 = s@W1.T + b1 with updates z1 += G1'@dh, G1' = -lr*(s@s.T + 1),
and evaluate queries from D = sum_t dh via q@W1.T + b1 - lr*(q@s.T + 1)@D.
Episodes are packed 5-per-group onto 125 partitions; 4 groups batched per
free-dim op; the 2nd MLP layer runs in a 125x25 cross form whose
off-diagonal blocks are masked to zero.
"""

import sys
import numpy as np

sys.path.insert(0, "/opt/trn_rl_repo")

from contextlib import ExitStack

import concourse.bass as bass  # noqa: E402
import concourse.mybir as mybir  # noqa: E402
import concourse.tile as tile  # noqa: E402
from concourse import bacc  # noqa: E402
from concourse.mybir import (  # noqa: E402
    AluOpType as alu,
    ActivationFunctionType as actf,
    AxisListType,
)

F32 = mybir.dt.float32
E, S, Qn, FEAT, HID, WAY = 4000, 25, 75, 64, 128, 5
ITERS, LR = 10, 0.01
G = 5            # episodes per partition-group (125 = 5*25 rows)
SG = G * S       # 125
B = 4            # groups batched per free-dim op
EB = G * B       # 20 episodes per round
NCORES = 8
EC = E // NCORES  # 500 episodes per core


# ---------------------------------------------------------------- host side
def _host_consts():
    ident = np.eye(128, dtype=np.float32)
    ee = np.repeat(np.arange(G), S)
    maskg = (-LR * (ee[:, None] == ee[None, :])).astype(np.float32)
    em5 = (ee[:, None] == np.arange(G)[None, :]).astype(np.float32)
    em5t = np.tile(em5 / S, (1, B))
    em5q = np.repeat(-LR * em5, Qn, axis=1).astype(np.float32)
    ind5 = (np.arange(G)[:, None] == ee[None, :]).astype(np.float32)
    eq = np.repeat(np.arange(G), Qn)
    indq = (np.arange(G)[:, None] == eq[None, :]).astype(np.float32)
    ones1 = np.ones((1, 128), dtype=np.float32)
    onesc = np.ones((128, 1), dtype=np.float32)
    return dict(ident=ident, maskg=maskg, em5t=em5t, em5q=em5q,
                ind5=ind5, indq=indq, ones1=ones1, onesc=onesc)


def _build_yc(tgt):
    """tgt [E, S] int -> yc [E//G, SG, G*WAY]: onehot/S on diagonal blocks."""
    nG = tgt.shape[0] // G
    t = tgt.reshape(nG, G, S)
    yc = np.zeros((nG, G, S, G, WAY), np.float32)
    oh = (t[..., None] == np.arange(WAY)).astype(np.float32) / S  # [nG,G,S,WAY]
    for e in range(G):
        yc[:, e, :, e, :] = oh[:, e]
    return yc.reshape(nG, SG, G * WAY)


# ---------------------------------------------------------------- device code
def _build_kernel(tc, EC):
    nc = tc.nc
    ctx = ExitStack()

    din = {}
    for name, shape in [
        ("sfT", [EC, FEAT, S]), ("qf", [EC, Qn, FEAT]),
        ("W1", [EC, HID, FEAT]), ("b1", [EC, HID]),
        ("W2", [EC, WAY, HID]), ("b2", [EC, WAY]),
        ("yc", [EC // G, SG, G * WAY]),
        ("ident", [128, 128]), ("maskg", [SG, SG]),
        ("em5t", [SG, B * G]), ("em5q", [SG, G * Qn]),
        ("ind5", [G, SG]), ("indq", [G, G * Qn]),
        ("ones1", [1, 128]), ("onesc", [128, 1]),
    ]:
        din[name] = nc.declare_dram_parameter(name, shape, F32, isOutput=False)
    dout = nc.declare_dram_parameter("out", [EC, Qn, WAY], F32, isOutput=True)

    cpool = ctx.enter_context(tc.tile_pool(name="consts", bufs=1))
    ident = cpool.tile([128, 128], F32)
    maskg = cpool.tile([SG, SG], F32)
    em5t = cpool.tile([SG, B * G], F32)
    em5q = cpool.tile([SG, G * Qn], F32)
    ind5 = cpool.tile([G, SG], F32)
    indq = cpool.tile([G, G * Qn], F32)
    ones1 = cpool.tile([1, 128], F32)
    onesc = cpool.tile([128, 1], F32)
    for t_, n_ in [(ident, "ident"), (maskg, "maskg"), (em5t, "em5t"),
                   (em5q, "em5q"), (ind5, "ind5"), (indq, "indq"),
                   (ones1, "ones1"), (onesc, "onesc")]:
        nc.sync.dma_start(out=t_[:], in_=din[n_][:])

    ld = ctx.enter_context(tc.tile_pool(name="loads", bufs=2))
    st = ctx.enter_context(tc.tile_pool(name="state", bufs=2))
    wk = ctx.enter_context(tc.tile_pool(name="work", bufs=2))
    ps = ctx.enter_context(tc.tile_pool(name="psum", bufs=1, space="PSUM"))

    for r in range(EC // EB):
        ep0 = r * EB

        # ---------------- round setup ----------------
        sT = ld.tile([FEAT, S * EB], F32, tag="sT")
        nc.sync.dma_start(out=sT[:].rearrange("f (e s) -> f e s", s=S),
                          in_=din["sfT"][ep0:ep0 + EB].transpose([1, 0, 2]))
        w1n = ld.tile([HID, FEAT * EB], F32, tag="w1n")
        nc.sync.dma_start(out=w1n[:].rearrange("h (e f) -> h e f", f=FEAT),
                          in_=din["W1"][ep0:ep0 + EB].transpose([1, 0, 2]))
        b1r = ld.tile([G, HID * B], F32, tag="b1r")
        nc.sync.dma_start(
            out=b1r[:].rearrange("e (g h) -> e g h", h=HID),
            in_=din["b1"][ep0:ep0 + EB]
                .rearrange("(g e) h -> g e h", g=B).transpose([1, 0, 2]))
        w2cat = st.tile([G * WAY, HID * B], F32, tag="w2cat")
        for g in range(B):
            nc.sync.dma_start(
                out=w2cat[:, g * HID:(g + 1) * HID],
                in_=din["W2"][ep0 + g * G:ep0 + (g + 1) * G]
                    .rearrange("e w h -> (e w) h"))
        b2row = st.tile([1, WAY * EB], F32, tag="b2row")
        nc.sync.dma_start(out=b2row[:], in_=din["b2"][ep0:ep0 + EB]
                          .rearrange("e w -> (e w)").unsqueeze(0))
        ycr = ld.tile([SG, G * WAY * B], F32, tag="ycr")
        nc.sync.dma_start(out=ycr[:].rearrange("p (b c) -> p b c", c=G * WAY),
                          in_=din["yc"][r * B:(r + 1) * B].transpose([1, 0, 2]))
        qn = ld.tile([Qn, FEAT * EB], F32, tag="qn")
        nc.sync.dma_start(out=qn[:].rearrange("q (e f) -> q e f", f=FEAT),
                          in_=din["qf"][ep0:ep0 + EB].transpose([1, 0, 2]))

        # W1^T / q^T via PE transposes (4 per psum bank)
        w1T = ld.tile([FEAT, HID * EB], F32, tag="w1T")
        for c in range(EB // 4):
            w1T_ps = ps.tile([FEAT, HID * 4], F32, tag="pa")
            for j in range(4):
                e = c * 4 + j
                nc.tensor.matmul(
                    w1T_ps[:, j * HID:(j + 1) * HID],
                    w1n[:, e * FEAT:(e + 1) * FEAT], ident[:HID, :HID],
                    is_transpose=True, start=(j == 0), stop=(j == 3),
                    skip_group_check=True)
            nc.vector.tensor_copy(
                w1T[:, c * 4 * HID:(c + 1) * 4 * HID], w1T_ps[:])
        qT = ld.tile([FEAT, Qn * EB], F32, tag="qT")
        for c in range(EB // 4):
            qT_ps = ps.tile([FEAT, 128 * 4], F32, tag="pb")
            for j in range(4):
                e = c * 4 + j
                nc.tensor.matmul(
                    qT_ps[:, j * 128:j * 128 + Qn],
                    qn[:, e * FEAT:(e + 1) * FEAT], ident[:Qn, :Qn],
                    is_transpose=True, start=(j == 0), stop=(j == 3),
                    skip_group_check=True)
            nc.vector.tensor_copy(
                qT[:, c * 4 * Qn:(c + 1) * 4 * Qn]
                  .rearrange("f (e q) -> f e q", q=Qn),
                qT_ps[:].rearrange("f (e p) -> f e p", p=128)[:, :, :Qn])

        # G1' = -lr*(s@s.T + 1) per episode (block-diagonal form)
        gram_ps = ps.tile([SG, SG * B], F32, tag="pc")
        for g in range(B):
            ssl = sT[:, g * SG:(g + 1) * SG]
            nc.tensor.matmul(gram_ps[:, g * SG:(g + 1) * SG], ssl, ssl,
                             start=(g == 0), stop=(g == B - 1),
                             skip_group_check=True)
        g1p = st.tile([SG, SG * B], F32, tag="g1p")
        for g in range(B):
            nc.vector.scalar_tensor_tensor(
                g1p[:, g * SG:(g + 1) * SG],
                gram_ps[:, g * SG:(g + 1) * SG],
                1.0, maskg[:], op0=alu.add, op1=alu.mult)

        # z1^0 (transposed build, then transpose back)
        z10T_ps = ps.tile([HID, SG * B], F32, tag="pd")
        for g in range(B):
            nc.tensor.matmul(
                z10T_ps[:, g * SG:(g + 1) * SG],
                b1r[:, g * HID:(g + 1) * HID], ind5[:],
                start=(g == 0), stop=False, skip_group_check=True)
            for ei in range(G):
                e = g * G + ei
                nc.tensor.matmul(
                    z10T_ps[:, g * SG + ei * S: g * SG + (ei + 1) * S],
                    w1T[:, e * HID:(e + 1) * HID],
                    sT[:, e * S:(e + 1) * S],
                    start=False, stop=(e == EB - 1), skip_group_check=True)
        z10T = wk.tile([HID, SG * B], F32, tag="z10T")
        nc.vector.tensor_copy(z10T[:], z10T_ps[:])
        z1_ps = ps.tile([SG, HID * B], F32, tag="pe")
        for g in range(B):
            nc.tensor.matmul(
                z1_ps[:, g * HID:(g + 1) * HID],
                z10T[:, g * SG:(g + 1) * SG], ident[:HID, :HID],
                is_transpose=True, start=(g == 0), stop=(g == B - 1),
                skip_group_check=True)
        z1 = st.tile([SG, HID * B], F32, tag="z1")
        nc.vector.tensor_copy(z1[:], z1_ps[:])

        # W2^T
        w2T_ps = ps.tile([HID, G * WAY * B], F32, tag="pf")
        for g in range(B):
            nc.tensor.matmul(
                w2T_ps[:, g * G * WAY:(g + 1) * G * WAY],
                w2cat[:, g * HID:(g + 1) * HID], ident[:G * WAY, :G * WAY],
                is_transpose=True, start=(g == 0), stop=(g == B - 1),
                skip_group_check=True)
        w2T = st.tile([HID, G * WAY * B], F32, tag="w2T")
        nc.vector.tensor_copy(w2T[:], w2T_ps[:])

        D = st.tile([SG, HID * B], F32, tag="D")
        nc.gpsimd.memset(D[:], 0.0)

        # ---------------- 10 SGD steps ----------------
        for t in range(ITERS):
            h = wk.tile([SG, HID * B], F32, tag="h")
            nc.scalar.activation(h[:], z1[:], actf.Relu)

            hT_ps = ps.tile([HID, SG * B], F32, tag="pa")
            for g in range(B):
                nc.tensor.matmul(
                    hT_ps[:, g * SG:(g + 1) * SG],
                    h[:, g * HID:(g + 1) * HID], ident[:SG, :SG],
                    is_transpose=True, start=(g == 0), stop=(g == B - 1),
                    skip_group_check=True)
            hT = wk.tile([HID, SG * B], F32, tag="hT")
            nc.vector.tensor_copy(hT[:], hT_ps[:])

            z2_ps = ps.tile([SG, G * WAY * B], F32, tag="pb")
            for g in range(B):
                nc.tensor.matmul(
                    z2_ps[:, g * G * WAY:(g + 1) * G * WAY],
                    hT[:, g * SG:(g + 1) * SG],
                    w2T[:, g * G * WAY:(g + 1) * G * WAY],
                    start=(g == 0), stop=False, skip_group_check=True)
                nc.tensor.matmul(
                    z2_ps[:, g * G * WAY:(g + 1) * G * WAY],
                    ones1[:1, :SG],
                    b2row[:, g * G * WAY:(g + 1) * G * WAY],
                    start=False, stop=(g == B - 1), skip_group_check=True)

            expz = wk.tile([SG, G * WAY * B], F32, tag="expz")
            nc.scalar.activation(expz[:], z2_ps[:], actf.Exp)
            Zs = wk.tile([SG, G * B], F32, tag="Zs")
            nc.vector.tensor_reduce(
                Zs[:].unsqueeze(2),
                expz[:].rearrange("p (a w) -> p a w", w=WAY),
                axis=AxisListType.X, op=alu.add)
            Zr = wk.tile([SG, G * B], F32, tag="Zr")
            nc.vector.reciprocal(Zr[:], Zs[:])
            Zrm = wk.tile([SG, G * B], F32, tag="Zrm")
            nc.vector.tensor_mul(Zrm[:], Zr[:], em5t[:])
            dlog = wk.tile([SG, G * WAY * B], F32, tag="dlog")
            nc.vector.tensor_mul(
                dlog[:].rearrange("p (a w) -> p a w", w=WAY),
                expz[:].rearrange("p (a w) -> p a w", w=WAY),
                Zrm[:].unsqueeze(2).broadcast_to([SG, G * B, WAY]))
            nc.vector.tensor_sub(dlog[:], dlog[:], ycr[:])

            dlT_ps = ps.tile([G * WAY, SG * B], F32, tag="pc")
            for g in range(B):
                nc.tensor.matmul(
                    dlT_ps[:, g * SG:(g + 1) * SG],
                    dlog[:, g * G * WAY:(g + 1) * G * WAY], ident[:SG, :SG],
                    is_transpose=True, start=(g == 0), stop=(g == B - 1),
                    skip_group_check=True)
            dlT = wk.tile([G * WAY, SG * B], F32, tag="dlT")
            nc.vector.tensor_copy(dlT[:], dlT_ps[:])

            dhp_ps = ps.tile([SG, HID * B], F32, tag="pd")
            for g in range(B):
                nc.tensor.matmul(
                    dhp_ps[:, g * HID:(g + 1) * HID],
                    dlT[:, g * SG:(g + 1) * SG],
                    w2cat[:, g * HID:(g + 1) * HID],
                    start=(g == 0), stop=(g == B - 1),
                    skip_group_check=True)
            msk = wk.tile([SG, HID * B], F32, tag="msk")
            nc.scalar.sign(msk[:], h[:])
            dh = wk.tile([SG, HID * B], F32, tag="dh")
            nc.vector.tensor_mul(dh[:], dhp_ps[:], msk[:])

            dw2_ps = ps.tile([G * WAY, HID * B], F32, tag="pe")
            for g in range(B):
                nc.tensor.matmul(
                    dw2_ps[:, g * HID:(g + 1) * HID],
                    dlog[:, g * G * WAY:(g + 1) * G * WAY],
                    h[:, g * HID:(g + 1) * HID],
                    start=(g == 0), stop=(g == B - 1),
                    skip_group_check=True)
            nc.vector.scalar_tensor_tensor(
                w2cat[:], dw2_ps[:], -LR, w2cat[:],
                op0=alu.mult, op1=alu.add)
            dw2T_ps = ps.tile([HID, G * WAY * B], F32, tag="pf")
            for g in range(B):
                nc.tensor.matmul(
                    dw2T_ps[:, g * G * WAY:(g + 1) * G * WAY],
                    h[:, g * HID:(g + 1) * HID],
                    dlog[:, g * G * WAY:(g + 1) * G * WAY],
                    start=(g == 0), stop=(g == B - 1),
                    skip_group_check=True)
            nc.vector.scalar_tensor_tensor(
                w2T[:], dw2T_ps[:], -LR, w2T[:],
                op0=alu.mult, op1=alu.add)
            db2_ps = ps.tile([1, WAY * EB], F32, tag="pg")
            nc.tensor.matmul(db2_ps[:], onesc[:SG, :], dlog[:],
                             start=True, stop=True)
            nc.vector.scalar_tensor_tensor(
                b2row[:], db2_ps[:], -LR, b2row[:],
                op0=alu.mult, op1=alu.add)

            z1d_ps = ps.tile([SG, HID * B], F32, tag="ph")
            for g in range(B):
                nc.tensor.matmul(
                    z1d_ps[:, g * HID:(g + 1) * HID],
                    g1p[:, g * SG:(g + 1) * SG],
                    dh[:, g * HID:(g + 1) * HID],
                    start=(g == 0), stop=(g == B - 1),
                    skip_group_check=True)
            nc.vector.tensor_add(z1[:], z1d_ps[:], z1[:])
            nc.vector.tensor_add(D[:], D[:], dh[:])

        # ---------------- query eval ----------------
        for g in range(B):
            e0 = g * G
            kq_ps = ps.tile([SG, G * Qn], F32, tag="pa")
            nc.tensor.matmul(kq_ps[:], sT[:, g * SG:(g + 1) * SG],
                             qT[:, e0 * Qn:(e0 + G) * Qn],
                             start=True, stop=True)
            kq1 = wk.tile([SG, G * Qn], F32, tag="kq1")
            nc.vector.scalar_tensor_tensor(
                kq1[:], kq_ps[:], 1.0, em5q[:],
                op0=alu.add, op1=alu.mult)

            zq_ps = ps.tile([HID, G * Qn], F32, tag="pb")
            nc.tensor.matmul(zq_ps[:], D[:, g * HID:(g + 1) * HID], kq1[:],
                             start=True, stop=False, skip_group_check=True)
            nc.tensor.matmul(zq_ps[:], b1r[:, g * HID:(g + 1) * HID], indq[:],
                             start=False, stop=False, skip_group_check=True)
            for ei in range(G):
                e = e0 + ei
                nc.tensor.matmul(
                    zq_ps[:, ei * Qn:(ei + 1) * Qn],
                    w1T[:, e * HID:(e + 1) * HID],
                    qT[:, e * Qn:(e + 1) * Qn],
                    start=False, stop=(ei == G - 1), skip_group_check=True)
            hq = wk.tile([HID, G * Qn], F32, tag="hq")
            nc.scalar.activation(hq[:], zq_ps[:], actf.Relu)

            o_ps = ps.tile([Qn, G * WAY], F32, tag="pc")
            for ei in range(G):
                nc.tensor.matmul(
                    o_ps[:, ei * WAY:(ei + 1) * WAY],
                    hq[:, ei * Qn:(ei + 1) * Qn],
                    w2T[:, (g * G + ei) * WAY:(g * G + ei + 1) * WAY],
                    start=(ei == 0), stop=False, skip_group_check=True)
                nc.tensor.matmul(
                    o_ps[:, ei * WAY:(ei + 1) * WAY],
                    ones1[:1, :Qn],
                    b2row[:, (g * G + ei) * WAY:(g * G + ei + 1) * WAY],
                    start=False, stop=(ei == G - 1), skip_group_check=True)
            osb = wk.tile([Qn, G * WAY], F32, tag="osb")
            nc.vector.tensor_copy(osb[:], o_ps[:])
            nc.sync.dma_start(
                out=dout[ep0 + e0:ep0 + e0 + G].transpose([1, 0, 2]),
                in_=osb[:].rearrange("q (e w) -> q e w", w=WAY))

    ctx.close()
    return din, dout


# ---------------------------------------------------------------- runner
_CACHE = {}


def _get_runner():
    if "run" in _CACHE:
        return _CACHE["run"]

    import jax
    from jax.sharding import Mesh, PartitionSpec
    from jax.experimental.shard_map import shard_map
    from concourse.bass2jax import (_bass_exec_p, install_neuronx_cc_hook,
                                    partition_id_tensor)

    install_neuronx_cc_hook()

    nc = bacc.Bacc("TRN2", target_bir_lowering=False, debug=False,
                   enable_asserts=False)
    with tile.TileContext(nc) as tc:
        _build_kernel(tc, EC)
    nc.compile()

    pid_name = (nc.partition_id_tensor.name
                if nc.partition_id_tensor is not None else None)
    in_names, out_names, out_avals, zero_shapes = [], [], [], []
    for alloc_ in nc.m.functions[0].allocations:
        if not isinstance(alloc_, mybir.MemoryLocationSet):
            continue
        name = alloc_.memorylocations[0].name
        if alloc_.kind == "ExternalInput":
            if name != pid_name:
                in_names.append(name)
        elif alloc_.kind == "ExternalOutput":
            out_names.append(name)
            shape = tuple(alloc_.tensor_shape)
            dtype = mybir.dt.np(alloc_.dtype)
            out_avals.append(jax.core.ShapedArray(shape, dtype))
            zero_shapes.append((shape, dtype))
    n_params = len(in_names)
    all_names = tuple(in_names + out_names
                      + ([pid_name] if pid_name else []))

    def _body(*args):
        operands = list(args)
        if pid_name is not None:
            operands.append(partition_id_tensor())
        outs = _bass_exec_p.bind(
            *operands,
            out_avals=tuple(out_avals),
            in_names=all_names,
            out_names=tuple(out_names),
            lowering_input_output_aliases=(),
            sim_require_finite=False,
            sim_require_nnan=False,
            nc=nc,
        )
        return tuple(outs)

    devices = jax.devices()[:NCORES]
    mesh = Mesh(np.asarray(devices), ("core",))
    n_outs = len(out_names)
    sharded = jax.jit(
        shard_map(
            _body, mesh=mesh,
            in_specs=(PartitionSpec("core"),) * (n_params + n_outs),
            out_specs=(PartitionSpec("core"),) * n_outs,
            check_rep=False,
        ),
        donate_argnums=tuple(range(n_params, n_params + n_outs)),
        keep_unused=True,
    )

    consts = _host_consts()
    const_g = {k: np.ascontiguousarray(np.tile(v, (NCORES, 1)))
               for k, v in consts.items()}
    _CACHE["run"] = (sharded, in_names, zero_shapes, const_g)
    return _CACHE["run"]


def kernel(query_feat, support_feat, support_targets, W1, b1, W2, b2):
    sharded, in_names, zero_shapes, const_g = _get_runner()

    sf = np.asarray(support_feat, np.float32)
    glob = {
        "sfT": np.ascontiguousarray(sf.transpose(0, 2, 1)),
        "qf": np.ascontiguousarray(np.asarray(query_feat, np.float32)),
        "W1": np.ascontiguousarray(np.asarray(W1, np.float32)),
        "b1": np.ascontiguousarray(np.asarray(b1, np.float32)),
        "W2": np.ascontiguousarray(np.asarray(W2, np.float32)),
        "b2": np.ascontiguousarray(np.asarray(b2, np.float32)),
        "yc": _build_yc(np.asarray(support_targets).astype(np.int64)),
    }
    glob.update(const_g)

    args = [glob[n] for n in in_names]
    args += [np.zeros((NCORES * s[0], *s[1:]), d) for s, d in zero_shapes]
    outs = sharded(*args)
    out = np.asarray(outs[0])          # [NCORES*EC, Qn, WAY]
    return out.reshape(E * Qn, WAY).astype(np.float32)
